# revision 36
# baseline (speedup 1.0000x reference)
"""Two-layer GCN block (PyG GCNConv x2) on 8 trn2 NeuronCores.

Per layer: out = D^-1/2 (A+I) D^-1/2 (h W) + b. The symmetric norm is
factorized: dis_s folds into the gather table (xu = dis*x for layer 1;
layer 1 writes v = dis*out1 for layer 2), dis_d is a per-tile lane scale
applied on the PSUM->SBUF copy. Dests are binned into 392 tiles of <=128
(by half-A source count then degree, so per-tile slot counts are tight),
dealt to 8 cores x 49 slots with one common schedule. Messages sit in a
slot grid [lane=dest, slot]: bulk-gathered in bf16 with gpsimd.dma_gather
(<=1024 idx per call, round-robin over 4 SWDGE queues), accumulated in
PSUM by identity matmuls, then (dis_d*acc)@W + b per tile. Both layers
share one table layout (assignment-ordered, piece-major) so one int16
index array serves both; layer-1 tiles are AllGathered in 4 pieces.
"""
import numpy as np

import sys
sys.path.insert(0, '/root/.axon_site')
sys.path.insert(0, '/opt/trn_rl_repo')

N = 50000
D = 64
DP = 128                     # padded feature row (bf16, 256B elems)
C = 8
NT = 49
NBINS = C * NT               # 392
NFULL = NBINS * 128          # 50176
P = 128
GROUP = 7
TB = [0, 7, 13, 19, 25, 31, 37, 43, 49]   # AllGather piece boundaries (slots)
VSPLIT_PIECE = 4
NPIECE = len(TB) - 1

PIECE_ROWS = [(TB[i + 1] - TB[i]) * 128 for i in range(NPIECE)]
REGION = np.concatenate([[0], np.cumsum([C * r for r in PIECE_ROWS])])
VSPLIT = int(REGION[VSPLIT_PIECE])

CHUNK = 8                    # blocks per dma_gather call (ring limit)

_compiled = None


SPLIT_SLOT = TB[VSPLIT_PIECE]          # 25


def _fill(order, slots, coreof, slotof, laneof):
    """Fill the (core, slot in `slots`, lane) positions with dests in
    `order`. Lane 127 of the first slot's tiles and of the last slot's
    tiles is reserved ghost (zero table row); shortfall lands at the end
    as extra ghosts."""
    pos = 0
    first, last = slots[0], slots[-1]
    for slot in slots:
        cap = 127 if slot in (first, last) else 128
        take = min(cap * C, len(order) - pos)
        if take <= 0:
            continue
        ds = order[pos:pos + take]
        i = np.arange(take)
        coreof[ds] = i % C
        slotof[ds] = slot
        laneof[ds] = i // C
        pos += take
    assert pos == len(order), (pos, len(order))


def _vrow(coreof, slotof, laneof):
    pieceof = np.digitize(slotof, TB[1:])
    pr = np.array(PIECE_ROWS, dtype=np.int64)
    tbl = np.array(TB[:NPIECE], dtype=np.int64)
    return (REGION[pieceof] + coreof * pr[pieceof]
            + (slotof - tbl[pieceof]) * 128 + laneof)


def _build(edge_index):
    row = np.asarray(edge_index[0], dtype=np.int64)
    col = np.asarray(edge_index[1], dtype=np.int64)

    deg = np.bincount(col, minlength=N).astype(np.float32) + 1.0
    dis = (1.0 / np.sqrt(deg)).astype(np.float32)

    # msgs: edges + self loops (dest d receives source s; scale folded away)
    msrc = np.concatenate([row, np.arange(N, dtype=np.int64)])
    mdst = np.concatenate([col, np.arange(N, dtype=np.int64)])

    # phase 1: half-A membership = top-degree nodes (fixed from here on)
    order = np.argsort(-deg, kind='stable')
    acap = C * (127 * 2 + 128 * (SPLIT_SLOT - 2))
    Aset = order[:acap]
    Bset = order[acap:]
    inA = np.zeros(N, dtype=bool)
    inA[Aset] = True
    # exact half-A source count per dest (self loop included)
    lA = np.bincount(mdst, weights=inA[msrc].astype(np.float64),
                     minlength=N).astype(np.int64)
    # phase 2: within each half, order by (lA, degree) for tight slot maxes
    key = lA * 1000 + deg.astype(np.int64)
    co = np.empty(N, dtype=np.int64)
    sl = np.empty(N, dtype=np.int64)
    la = np.empty(N, dtype=np.int64)
    _fill(Aset[np.argsort(-key[Aset], kind='stable')],
          list(range(0, SPLIT_SLOT)), co, sl, la)
    _fill(Bset[np.argsort(-key[Bset], kind='stable')],
          list(range(SPLIT_SLOT, NT)), co, sl, la)
    vr = _vrow(co, sl, la)

    # final per-msg placement
    half = (vr[msrc] >= VSPLIT).astype(np.int64)
    idxs = vr[msrc] - half * VSPLIT

    # per (core, slot, lane, half) counts -> slot positions
    lane_key = ((co[mdst] * NT + sl[mdst]) * 128 + la[mdst]) * 2 + half
    perm = np.argsort(lane_key, kind='stable')
    ks, ids = lane_key[perm], idxs[perm]
    bounds = np.searchsorted(ks, np.arange(C * NT * 128 * 2 + 1))
    cnt = np.diff(bounds).reshape(C, NT, 128, 2)
    nb = cnt.max(axis=(0, 2))                     # [NT, 2] common schedule
    NB = int(nb.sum())

    blk_start = np.zeros((NT, 2), dtype=np.int64)
    ccols = []
    pos = 0
    for g in range(0, NT, GROUP):
        tiles = list(range(g, min(g + GROUP, NT)))
        for h in (0, 1):
            n_here = 0
            for t in tiles:
                blk_start[t, h] = pos + n_here
                n_here += int(nb[t, h])
            ccols.append((pos, n_here, h, tiles))
            pos += n_here
    assert pos == NB

    # dummy (padding) indices: guaranteed zero rows per half, per core
    dumA = _vrow(np.arange(C), np.zeros(C, dtype=np.int64),
                 np.full(C, 127, dtype=np.int64))
    dumB = _vrow(np.arange(C), np.full(C, 48, dtype=np.int64),
                 np.full(C, 127, dtype=np.int64)) - VSPLIT
    assert dumA.max() < VSPLIT and dumB.min() >= 0

    COLS = NB * 8
    idx_arr = np.zeros((C, P, COLS), dtype=np.int16)
    disd_arr = np.zeros((C, P, NT), dtype=np.float32)
    disd_arr[co, la, sl] = dis

    # vectorized slot placement: msg rank within its (core,slot,lane,half)
    swi = np.arange(ks.size) - bounds[ks]
    mk = ks >> 1
    mh = ks & 1
    mp = mk % 128
    mt = (mk // 128) % NT
    mcore = mk // (128 * NT)
    flat_pos = (blk_start[mt, mh] + swi) * 128 + mp
    # half of each block position (for dummy fill)
    halfof = np.zeros(NB, dtype=np.int64)
    for t in range(NT):
        halfof[blk_start[t, 1]:blk_start[t, 1] + int(nb[t, 1])] = 1
    hrep = np.repeat(halfof, 128)
    for k in range(C):
        flat = np.empty(NB * 128, dtype=np.int64)
        flat[:] = -1
        mkk = mcore == k
        flat[flat_pos[mkk]] = ids[mkk]
        mask = flat < 0
        flat[mask & (hrep == 0)] = dumA[k]
        flat[mask & (hrep == 1)] = dumB[k]
        w = flat.reshape(-1, 16).T.astype(np.int16)
        idx_arr[k] = np.tile(w, (8, 1))
    return dict(nb=nb, NB=NB, COLS=COLS, idx_arr=idx_arr, disd_arr=disd_arr,
                blk_start=blk_start, ccols=ccols,
                co=co, sl=sl, la=la, vr=vr, dis=dis)


def kernel(x, edge_index, W1, b1, W2, b2):
    import concourse.bass as bass  # noqa: F401
    import concourse.bacc as bacc
    import concourse.mybir as mybir
    from concourse import tile
    from concourse import library_config
    from concourse.bass_utils import run_bass_kernel_spmd
    import ml_dtypes

    x = np.asarray(x, dtype=np.float32)
    W1 = np.asarray(W1, dtype=np.float32)
    W2 = np.asarray(W2, dtype=np.float32)
    b1 = np.asarray(b1, dtype=np.float32)
    b2 = np.asarray(b2, dtype=np.float32)

    S = _build(edge_index)
    co, sl, la, vr, dis = S['co'], S['sl'], S['la'], S['vr'], S['dis']
    NB, COLS, nb, blk_start, ccols = (S['NB'], S['COLS'], S['nb'],
                                      S['blk_start'], S['ccols'])

    # xu table in assignment order, bf16, padded to 128 features
    xu = np.zeros((NFULL, DP), dtype=ml_dtypes.bfloat16)
    xu[vr, :D] = (dis[:, None] * x).astype(ml_dtypes.bfloat16)

    ident_np = np.eye(P, dtype=np.float32)
    identb_np = np.eye(P, dtype=ml_dtypes.bfloat16)

    NBG = max(ccols[i][1] + ccols[i + 1][1] for i in range(0, len(ccols), 2))

    nc = bacc.Bacc(None, target_bir_lowering=False, num_swdge_queues=4)
    dt = mybir.dt
    xup = nc.declare_dram_parameter("xup", [NFULL, DP], dt.bfloat16, isOutput=False)
    identp = nc.declare_dram_parameter("identp", [P, P], dt.float32, isOutput=False)
    identbp = nc.declare_dram_parameter("identbp", [P, P], dt.bfloat16, isOutput=False)
    idxp = nc.declare_dram_parameter("idxp", [P, COLS], dt.int16, isOutput=False)
    disdp = nc.declare_dram_parameter("disdp", [P, NT], dt.float32, isOutput=False)
    w1p = nc.declare_dram_parameter("w1p", [D, D], dt.bfloat16, isOutput=False)
    w2p = nc.declare_dram_parameter("w2p", [D, D], dt.bfloat16, isOutput=False)
    b1p = nc.declare_dram_parameter("b1p", [D, 1], dt.float32, isOutput=False)
    b2p = nc.declare_dram_parameter("b2p", [D, 1], dt.float32, isOutput=False)
    out_sh = nc.declare_dram_parameter("out_sh", [NT * 128, D], dt.float32,
                                       isOutput=True)
    import os
    DBG = os.environ.get("KDBG") == "1"
    DBG2 = os.environ.get("KDBG") == "2"
    if DBG:
        dbg_v = nc.declare_dram_parameter("dbg_v", [NT * 128, D], dt.bfloat16,
                                          isOutput=True)
    if DBG2:
        dbg_vf = nc.declare_dram_parameter("dbg_vf", [NFULL, DP], dt.bfloat16,
                                           isOutput=True)

    v_shp = [nc.dram_tensor(f"v_sh{i}", [PIECE_ROWS[i], DP], dt.bfloat16)
             for i in range(NPIECE)]
    vfull = nc.dram_tensor("vfull", [NFULL, DP], dt.bfloat16,
                           addr_space="Shared")

    rg = [list(range(C))]
    Copy = mybir.ActivationFunctionType.Copy
    Ident = mybir.ActivationFunctionType.Identity

    with tile.TileContext(nc) as tc:
        with tc.tile_pool(name="const", bufs=1) as cp, \
             tc.tile_pool(name="gp", bufs=3) as gpool, \
             tc.tile_pool(name="ep", bufs=4) as ep, \
             tc.tile_pool(name="psA", bufs=2, space="PSUM") as psA, \
             tc.tile_pool(name="psB1", bufs=2, space="PSUM") as psB1, \
             tc.tile_pool(name="psB2", bufs=2, space="PSUM") as psB2, \
             tc.tile_pool(name="psC", bufs=2, space="PSUM") as psC:

            nc.gpsimd.load_library(library_config.mlp)

            ident = cp.tile([P, P], dt.float32)
            nc.sync.dma_start(out=ident[:], in_=identp[:, :])
            identb = cp.tile([P, P], dt.bfloat16)
            nc.sync.dma_start(out=identb[:], in_=identbp[:, :])
            w1t = cp.tile([D, D], dt.bfloat16)
            nc.sync.dma_start(out=w1t[:], in_=w1p[:, :])
            w2t = cp.tile([D, D], dt.bfloat16)
            nc.sync.dma_start(out=w2t[:], in_=w2p[:, :])
            b1t = cp.tile([D, 1], dt.float32)
            nc.sync.dma_start(out=b1t[:], in_=b1p[:, :])
            b2t = cp.tile([D, 1], dt.float32)
            nc.sync.dma_start(out=b2t[:], in_=b2p[:, :])
            idxt = cp.tile([P, COLS], dt.int16)
            h0 = COLS // 2
            nc.sync.dma_start(out=idxt[:, :h0], in_=idxp[:, :h0])
            nc.sync.dma_start(out=idxt[:, h0:], in_=idxp[:, h0:])
            disdt = cp.tile([P, NT], dt.float32)
            nc.sync.dma_start(out=disdt[:], in_=disdp[:, :])

            qctr = [0]
            ag_insts = [None] * NPIECE

            def layer(tab, wt, bt, scale_out, dest_of, post_tile=None,
                      gather_deps=None):
                colpos = {}
                cpos = 0
                for (b0, nbl, h, tiles) in ccols:
                    colpos[(tiles[0], h)] = cpos
                    cpos += nbl * 8

                def gcalls(base_lo, c0, goff, nblk, gbuf, h):
                    from concourse.tile_rust import add_dep_helper
                    for off in range(0, nblk, CHUNK):
                        m = min(CHUNK, nblk - off)
                        g = nc.gpsimd.dma_gather(
                            gbuf[:, goff + off:goff + off + m, :],
                            tab[base_lo[0]:base_lo[1], :],
                            idxt[:, c0 + off * 8:c0 + (off + m) * 8],
                            m * 128, m * 128, DP, queue_num=qctr[0] % 4)
                        qctr[0] += 1
                        if gather_deps is not None:
                            for cc in gather_deps(h):
                                add_dep_helper(
                                    g.ins, cc.ins,
                                    reason="L2 gather waits on AllGather")

                groups = [(ccols[i], ccols[i + 1])
                          for i in range(0, len(ccols), 2)]
                for (cA, cB) in groups:
                    b0A, nA, _, tiles = cA
                    b0B, nB_, _, _ = cB
                    gbuf = gpool.tile([P, NBG, DP], dt.bfloat16, tag="g")
                    if nA > 0:
                        gcalls((0, VSPLIT), colpos[(tiles[0], 0)], 0, nA,
                               gbuf, 0)
                    if nB_ > 0:
                        gcalls((VSPLIT, NFULL), colpos[(tiles[0], 1)],
                               nA, nB_, gbuf, 1)
                    for t in tiles:
                        nblk = int(nb[t, 0] + nb[t, 1])
                        acc = psA.tile([P, D], dt.float32)
                        j = 0
                        for h in (0, 1):
                            bs = int(blk_start[t, h]) - b0A
                            for i in range(int(nb[t, h])):
                                nc.tensor.matmul(acc[:], lhsT=identb[:],
                                                 rhs=gbuf[:, bs + i, 0:D],
                                                 start=(j == 0),
                                                 stop=(j == nblk - 1))
                                j += 1
                        # asb = dis_d * acc  (bf16)
                        asb = ep.tile([P, D], dt.bfloat16, tag="a")
                        nc.scalar.activation(out=asb[:], in_=acc[:], func=Copy,
                                             scale=disdt[:, t:t + 1])
                        tr1 = psB1.tile([D, P], dt.bfloat16)
                        nc.tensor.transpose(tr1[:], asb[:], identb[:])
                        ct = ep.tile([D, P], dt.bfloat16, tag="c")
                        nc.scalar.activation(out=ct[:], in_=tr1[:], func=Copy)
                        pv = psC.tile([D, P], dt.float32)
                        nc.tensor.matmul(pv[:], lhsT=wt[:], rhs=ct[:],
                                         start=True, stop=True)
                        vt = ep.tile([D, P], dt.bfloat16, tag="v")
                        nc.scalar.activation(out=vt[:], in_=pv[:], func=Ident,
                                             bias=bt[:, 0:1])
                        tr2 = psB2.tile([P, D], dt.bfloat16)
                        nc.tensor.matmul(tr2[:], lhsT=vt[:],
                                         rhs=identb[:D, :D],
                                         is_transpose=True)
                        dest_of(t, tr2)
                        if post_tile is not None:
                            post_tile(t)

            def v_dest(t, tr2):
                p = 0
                while t >= TB[p + 1]:
                    p += 1
                off = (t - TB[p]) * P
                vsb = ep.tile([P, D], dt.bfloat16, tag="o")
                # v = dis_d * out1   (ghost lanes -> 0)
                nc.scalar.activation(out=vsb[:], in_=tr2[:], func=Copy,
                                     scale=disdt[:, t:t + 1])
                nc.sync.dma_start(out=v_shp[p][off:off + P, 0:D], in_=vsb[:])
                if DBG:
                    nc.sync.dma_start(out=dbg_v[t * P:(t + 1) * P, :],
                                      in_=vsb[:])

            def o_dest(t, tr2):
                vsb = ep.tile([P, D], dt.float32, tag="o2")
                nc.scalar.activation(out=vsb[:], in_=tr2[:], func=Copy)
                nc.sync.dma_start(out=out_sh[t * P:(t + 1) * P, :], in_=vsb[:])

            def fire_ag(t):
                for i in range(NPIECE):
                    if t == TB[i + 1] - 1:
                        lo, hi = int(REGION[i]), int(REGION[i + 1])
                        ag_insts[i] = nc.gpsimd.collective_compute(
                            "AllGather", mybir.AluOpType.bypass,
                            replica_groups=rg,
                            ins=[v_shp[i][:]], outs=[vfull[lo:hi, :]])

            layer(xup, w1t, b1t, True, v_dest, fire_ag)
            if DBG2:
                for i in range(0, NFULL, 1792):
                    hi = min(i + 1792, NFULL)
                    st = ep.tile([P, (1792 // P) * DP], dt.bfloat16, tag="dbg")
                    nc.sync.dma_start(out=st[:, :(hi - i) // P * DP],
                                      in_=vfull[i:hi, :])
                    nc.sync.dma_start(out=dbg_vf[i:hi, :],
                                      in_=st[:, :(hi - i) // P * DP])
            layer(vfull, w2t, b2t, False, o_dest,
                  gather_deps=lambda h: (ag_insts[0:VSPLIT_PIECE] if h == 0
                                         else ag_insts[VSPLIT_PIECE:]))

    nc.compile()

    in_maps = []
    for k in range(C):
        in_maps.append({
            "xup": xu, "identp": ident_np,
            "identbp": identb_np,
            "idxp": S['idx_arr'][k], "disdp": S['disd_arr'][k],
            "w1p": W1.astype(ml_dtypes.bfloat16),
            "w2p": W2.astype(ml_dtypes.bfloat16),
            "b1p": b1.reshape(D, 1), "b2p": b2.reshape(D, 1),
        })
    global _compiled
    _compiled = (nc, in_maps)
    res = run_bass_kernel_spmd(nc, in_maps, list(range(C)))
    allout = np.stack([res.results[k]["out_sh"] for k in range(C)])
    out = allout[co, sl * 128 + la]
    return np.ascontiguousarray(out)


def profile_last():
    """Re-run the last compiled program with NTFF tracing; returns exec ns."""
    from concourse.bass_utils import run_bass_kernel_spmd
    assert _compiled is not None
    nc, in_maps = _compiled
    r = run_bass_kernel_spmd(nc, in_maps, list(range(C)), trace=True)
    return r.exec_time_ns
